# revision 10
# baseline (speedup 1.0000x reference)
"""TAGConv GNN classifier on 8 Trainium2 NeuronCores.

Sharding: nodes split into 8 contiguous slices (6250/core, padded to 6272);
edges live on the core that owns their dst. Each hop: every core gathers
src rows from a replicated norm-prescaled node table in HBM (dma_gather,
int16 indices -> split-table trick), segment-sums them into its dst slice
with one-hot matmuls on TensorE (PSUM accumulation), rescales by norm, and
all-gathers its slice of the next table. Readout partial sums per graph are
all-reduced, then every core computes the (identical) logits.
"""
import os

import numpy as np

import concourse.bass as bass
import concourse.bacc as bacc
import concourse.mybir as mybir
import concourse.tile as tile
from concourse import bass_utils

N, E, G = 50000, 800000, 128
F = 128                      # IN_DIM == HID
CLASSES = 10
HOPS, HLAYERS = 2, 2         # 3 TAGConv layers total
NCORES = 8


def configure(n, e):
    """Derived sizes; module-level so debug harnesses can shrink the problem."""
    global N, E, PER, GRP, NPAD, NT, HALF
    N, E = n, e
    PER = N // NCORES            # real nodes per core
    GRP = (PER + 127) // 128     # dst groups of 128 per core
    NPAD = GRP * 128             # padded nodes per core
    NT = NCORES * NPAD           # padded total
    HALF = NT // 2               # int16-safe split of the node table


configure(N, E)

FP = mybir.dt.float32
I16 = mybir.dt.int16


def _prep_edges(src, dst):
    """Per-core gather-index + one-hot-slot tables, SPMD-uniform shapes."""
    src = src.astype(np.int64)
    dst = dst.astype(np.int64)
    core = dst // PER
    local = dst - core * PER
    grp = local // 128
    slot = local % 128
    ps = (src // PER) * NPAD + (src % PER)          # padded global src id
    half = (ps >= HALF).astype(np.int64)
    idxv = ps - half * HALF                          # int16-safe index

    key = (core * GRP + grp) * 2 + half
    order = np.argsort(key, kind="stable")
    cnt = np.bincount(key, minlength=NCORES * GRP * 2).reshape(NCORES, GRP, 2)
    CA = np.maximum(1, -(-cnt[:, :, 0].max(axis=0) // 128)).astype(int)
    CB = np.maximum(1, -(-cnt[:, :, 1].max(axis=0) // 128)).astype(int)
    nch = CA + CB                                    # chunks per group
    choff = np.concatenate([[0], np.cumsum(nch)]).astype(int)
    NCH = int(choff[-1])
    TOT = NCH * 128

    idx16 = np.zeros((NCORES, TOT), np.int16)
    slotv = np.full((NCORES, TOT), -1.0, np.float32)
    sidx = idxv[order]
    sslot = slot[order]
    starts = np.concatenate([[0], np.cumsum(cnt.reshape(-1))]).astype(int)
    for c in range(NCORES):
        for g in range(GRP):
            base = choff[g] * 128
            for h, off in ((0, base), (1, base + CA[g] * 128)):
                k = (c * GRP + g) * 2 + h
                n = int(cnt[c, g, h])
                s0 = starts[k]
                idx16[c, off : off + n] = sidx[s0 : s0 + n]
                slotv[c, off : off + n] = sslot[s0 : s0 + n]

    idx_w = np.stack([np.tile(idx16[c].reshape(-1, 16).T, (8, 1)) for c in range(NCORES)])
    slot_cols = np.stack([slotv[c].reshape(NCH, 128).T for c in range(NCORES)])
    return idx_w, slot_cols, CA, CB, choff, NCH, TOT


def _build_program(CA, CB, choff, NCH, TOT):
    STAGE = os.environ.get("KSTAGE", "full")
    ORDER = ["deg", "t0", "ag0", "hop1", "aghop", "hop2", "layer0", "full"]
    LVL = ORDER.index(STAGE)
    nc = bacc.Bacc("TRN2", target_bir_lowering=False, debug=False, num_devices=NCORES)
    RG = [list(range(NCORES))]
    W16 = TOT // 16

    x_d = nc.dram_tensor("x_loc", [NPAD, F], FP, kind="ExternalInput")
    idx_d = nc.dram_tensor("idx_w", [128, W16], I16, kind="ExternalInput")
    slot_d = nc.dram_tensor("slot_cols", [128, NCH], FP, kind="ExternalInput")
    gslot_d = nc.dram_tensor("gslot", [128, GRP], FP, kind="ExternalInput")
    valid_d = nc.dram_tensor("valid", [128, GRP], FP, kind="ExternalInput")
    w_d = [nc.dram_tensor(f"w{l}", [(HOPS + 1) * F, F], FP, kind="ExternalInput")
           for l in range(HLAYERS + 1)]
    b_d = nc.dram_tensor("b_cols", [128, HLAYERS + 1], FP, kind="ExternalInput")
    wc_d = nc.dram_tensor("wc", [F, CLASSES], FP, kind="ExternalInput")
    bcr_d = nc.dram_tensor("bc_rep", [128, CLASSES], FP, kind="ExternalInput")
    out_d = nc.dram_tensor("out", [G, CLASSES], FP, kind="ExternalOutput")

    with tile.TileContext(nc) as tc:
        with (
            tc.tile_pool(name="const", bufs=1) as cp,
            tc.tile_pool(name="work", bufs=2) as wp,
            tc.tile_pool(name="psmm", bufs=3, space="PSUM") as pmm,
            tc.tile_pool(name="pstr", bufs=2, space="PSUM") as ptr,
            tc.tile_pool(name="psro", bufs=2, space="PSUM") as pro,
            tc.tile_pool(name="dram", bufs=1, space="DRAM") as dp,
        ):
            # ---- persistent tiles ----
            idx_t = cp.tile([128, W16], I16)
            slot_t = cp.tile([128, NCH], FP)
            gslot_t = cp.tile([128, GRP], FP)
            valid_t = cp.tile([128, GRP], FP)
            iota_t = cp.tile([128, 128], FP)
            ident_t = cp.tile([128, 128], FP)
            ones_t = cp.tile([128, 1], FP)
            normc_t = cp.tile([128, GRP], FP)
            w_t = [cp.tile([128, HOPS + 1, F], FP, name=f"w{l}_t", tag=f"w{l}")
                   for l in range(HLAYERS + 1)]
            b_t = cp.tile([128, HLAYERS + 1], FP)
            wc_t = cp.tile([F, CLASSES], FP)
            bcr_t = cp.tile([128, CLASSES], FP)
            f0T = cp.tile([128, GRP * 128], FP)   # feat-major [f, i] per group
            f1T = cp.tile([128, GRP * 128], FP)
            f2T = cp.tile([128, GRP * 128], FP)
            roacc_t = cp.tile([128, F + 1], FP)
            ro2_t = cp.tile([128, F + 1], FP)
            cnt_t = cp.tile([128, 1], FP)
            rcp_t = cp.tile([128, 1], FP)
            hg_t = cp.tile([128, F], FP)
            hgT_t = cp.tile([F, 128], FP)
            logit_t = cp.tile([128, CLASSES], FP)

            T_in = dp.tile([NT, F], FP)
            T_hop = dp.tile([NT, F], FP)
            ag_in = dp.tile([NPAD, F], FP)
            ar_in = dp.tile([128, F + 1], FP)
            ar_out = dp.tile([128, F + 1], FP)

            # ---- constants ----
            nc.sync.dma_start(idx_t[:], idx_d[:, :])
            nc.sync.dma_start(slot_t[:], slot_d[:, :])
            nc.sync.dma_start(gslot_t[:], gslot_d[:, :])
            nc.sync.dma_start(valid_t[:], valid_d[:, :])
            for l in range(HLAYERS + 1):
                for k in range(HOPS + 1):
                    nc.sync.dma_start(w_t[l][:, k, :], w_d[l][k * 128 : (k + 1) * 128, :])
            nc.sync.dma_start(b_t[:], b_d[:, :])
            nc.sync.dma_start(wc_t[:], wc_d[:, :])
            nc.sync.dma_start(bcr_t[:], bcr_d[:, :])

            nc.gpsimd.iota(iota_t[:], pattern=[[1, 128]], base=0, channel_multiplier=0,
                           allow_small_or_imprecise_dtypes=True)
            icol_t = cp.tile([128, 1], FP)
            nc.gpsimd.iota(icol_t[:], pattern=[[0, 1]], base=0, channel_multiplier=1,
                           allow_small_or_imprecise_dtypes=True)
            nc.vector.tensor_tensor(ident_t[:], icol_t[:].broadcast_to([128, 128]),
                                    iota_t[:], mybir.AluOpType.is_equal)
            nc.vector.memset(ones_t[:], 1.0)
            nc.vector.memset(roacc_t[:], 0.0)

            def bail():
                nc.vector.tensor_copy(logit_t[:], iota_t[:, :CLASSES])
                nc.sync.dma_start(out_d[:, :], logit_t[:])

            def onehot_all(g):
                """[128e, nch, 128j] one-hot tile for group g (one DVE op)."""
                nch = int(CA[g] + CB[g])
                c0 = int(choff[g])
                oh = wp.tile([128, int(max(CA + CB)), 128], FP, name="oh", tag="oh")
                nc.vector.tensor_tensor(
                    oh[:, :nch, :],
                    slot_t[:, c0 : c0 + nch].unsqueeze(2).broadcast_to([128, nch, 128]),
                    iota_t[:].unsqueeze(1).broadcast_to([128, nch, 128]),
                    mybir.AluOpType.is_equal,
                )
                return oh, nch

            # ---- degree / norm pass ----
            for g in range(GRP):
                oh, nch = onehot_all(g)
                dps = pmm.tile([128, 128], FP, name="dps", tag="mm")
                for c in range(nch):
                    nc.tensor.matmul(dps[:, 0:1], oh[:, c, :], ones_t[:],
                                     start=(c == 0), stop=(c == nch - 1))
                dmx = wp.tile([128, 1], FP, name="dmx", tag="dmx")
                nc.vector.tensor_scalar_max(dmx[:], dps[:, 0:1], 1.0)
                drc = wp.tile([128, 1], FP, name="drc", tag="drc")
                nc.vector.reciprocal(drc[:], dmx[:])
                nc.scalar.activation(normc_t[:, g : g + 1], drc[:],
                                     mybir.ActivationFunctionType.Sqrt)
            STOP = LVL <= ORDER.index("deg")
            if STOP:
                bail()

            # ---- T0 = x * norm ; f0T = x^T ----
            for g in range(GRP) if not STOP else []:
                gs = slice(g * 128, (g + 1) * 128)
                xt = wp.tile([128, F], FP, name="xt", tag="xt")
                nc.sync.dma_start(xt[:], x_d[gs, :])
                t0 = wp.tile([128, F], FP, name="t0", tag="tn")
                nc.vector.tensor_tensor(t0[:], xt[:],
                                        normc_t[:, g : g + 1].broadcast_to([128, F]),
                                        mybir.AluOpType.mult)
                nc.sync.dma_start(ag_in[gs, :], t0[:])
                pt = ptr.tile([128, 128], FP, name="pt", tag="tr")
                nc.tensor.transpose(pt[:], xt[:], ident_t[:])
                nc.vector.tensor_copy(f0T[:, gs], pt[:])
            if not STOP and LVL <= ORDER.index("t0"):
                bail()
                STOP = True
            if not STOP:
                nc.gpsimd.collective_compute(
                    "AllGather", mybir.AluOpType.bypass, replica_groups=RG,
                    ins=[ag_in.opt()], outs=[T_in.opt()])
            if not STOP and LVL <= ORDER.index("ag0"):
                bail()
                STOP = True

            def hop(src_tbl, fT, make_table):
                """One SpMM hop: gather -> one-hot segsum -> scale; optionally
                also emit next scaled table slice into ag_in."""
                KSUB = os.environ.get("KSUB", "full")
                for g in range(GRP):
                    gs = slice(g * 128, (g + 1) * 128)
                    ca, cb = int(CA[g]), int(CB[g])
                    nch = ca + cb
                    c0 = int(choff[g])
                    vb = wp.tile([128, int(max(CA + CB)), 128], FP, name="vb", tag="vb")
                    colA = c0 * 8
                    colB = colA + ca * 8
                    if KSUB in ("full", "gath"):
                        nc.gpsimd.dma_gather(
                            vb[:, 0:ca, :], src_tbl[:, :], idx_t[:, colA : colA + ca * 8],
                            ca * 128, ca * 128, F, single_packet=False)
                        nc.gpsimd.dma_gather(
                            vb[:, ca:nch, :], src_tbl[HALF:, :], idx_t[:, colB : colB + cb * 8],
                            cb * 128, cb * 128, F, single_packet=False)
                    else:
                        nc.vector.memset(vb[:], 0.0)
                    fn = wp.tile([128, F], FP, name="fn", tag="fn")
                    if KSUB == "gath":
                        nc.vector.tensor_copy(fn[:], vb[:, 0, :])
                    else:
                        oh, _ = onehot_all(g)
                        ps = pmm.tile([128, 128], FP, name="ps", tag="mm")
                        for c in range(nch):
                            nc.tensor.matmul(ps[:], oh[:, c, :], vb[:, c, :],
                                             start=(c == 0), stop=(c == nch - 1))
                        nc.vector.tensor_tensor(fn[:], ps[:],
                                                normc_t[:, g : g + 1].broadcast_to([128, F]),
                                                mybir.AluOpType.mult)
                    if make_table:
                        tn = wp.tile([128, F], FP, name="tn", tag="tn")
                        nc.vector.tensor_tensor(tn[:], fn[:],
                                                normc_t[:, g : g + 1].broadcast_to([128, F]),
                                                mybir.AluOpType.mult)
                        nc.sync.dma_start(ag_in[gs, :], tn[:])
                    pt = ptr.tile([128, 128], FP, name="pt2", tag="tr")
                    nc.tensor.transpose(pt[:], fn[:], ident_t[:])
                    nc.vector.tensor_copy(fT[:, gs], pt[:])

            for l in range(HLAYERS + 1) if not STOP else []:
                hop(T_in, f1T, make_table=True)
                if l == 0 and LVL <= ORDER.index("hop1"):
                    bail()
                    STOP = True
                    break
                nc.gpsimd.collective_compute(
                    "AllGather", mybir.AluOpType.bypass, replica_groups=RG,
                    ins=[ag_in.opt()], outs=[T_hop.opt()])
                if l == 0 and LVL <= ORDER.index("aghop"):
                    bail()
                    STOP = True
                    break
                hop(T_hop, f2T, make_table=False)
                if l == 0 and LVL <= ORDER.index("hop2"):
                    bail()
                    STOP = True
                    break
                fTs = [f0T, f1T, f2T]
                for g in range(GRP):
                    gs = slice(g * 128, (g + 1) * 128)
                    ph = pmm.tile([128, 128], FP, name="ph", tag="mm")
                    for k in range(HOPS + 1):
                        nc.tensor.matmul(ph[:], w_t[l][:, k, :], fTs[k][:, gs],
                                         start=(k == 0), stop=(k == HOPS))
                    nc.scalar.activation(f0T[:, gs], ph[:],
                                         mybir.ActivationFunctionType.Relu,
                                         bias=b_t[:, l : l + 1])
                    pt = ptr.tile([128, 128], FP, name="pt3", tag="tr")
                    nc.tensor.transpose(pt[:], f0T[:, gs], ident_t[:])
                    if l < HLAYERS:
                        tn = wp.tile([128, F], FP, name="tn2", tag="tn")
                        nc.vector.tensor_tensor(tn[:], pt[:],
                                                normc_t[:, g : g + 1].broadcast_to([128, F]),
                                                mybir.AluOpType.mult)
                        nc.sync.dma_start(ag_in[gs, :], tn[:])
                    else:
                        rr = wp.tile([128, F + 1], FP, name="rr", tag="rr")
                        nc.vector.tensor_copy(rr[:, 0:F], pt[:])
                        nc.vector.tensor_copy(rr[:, F : F + 1], valid_t[:, g : g + 1])
                        og = wp.tile([128, 128], FP, name="og", tag="og")
                        nc.vector.tensor_tensor(
                            og[:], gslot_t[:, g : g + 1].broadcast_to([128, 128]),
                            iota_t[:], mybir.AluOpType.is_equal)
                        pr = pro.tile([128, F + 1], FP, name="pr", tag="ro")
                        nc.tensor.matmul(pr[:], og[:], rr[:], start=True, stop=True)
                        nc.vector.tensor_tensor(roacc_t[:], roacc_t[:], pr[:],
                                                mybir.AluOpType.add)
                if l < HLAYERS:
                    nc.gpsimd.collective_compute(
                        "AllGather", mybir.AluOpType.bypass, replica_groups=RG,
                        ins=[ag_in.opt()], outs=[T_in.opt()])
                if l == 0 and LVL <= ORDER.index("layer0"):
                    bail()
                    STOP = True
                    break

            # ---- readout: all-reduce partial sums, mean, classify ----
            if not STOP:
                nc.sync.dma_start(ar_in[:, :], roacc_t[:])
                nc.gpsimd.collective_compute(
                    "AllReduce", mybir.AluOpType.add, replica_groups=RG,
                    ins=[ar_in.opt()], outs=[ar_out.opt()])
                nc.sync.dma_start(ro2_t[:], ar_out[:, :])
                nc.vector.tensor_scalar_max(cnt_t[:], ro2_t[:, F : F + 1], 1.0)
                nc.vector.reciprocal(rcp_t[:], cnt_t[:])
                nc.vector.tensor_tensor(hg_t[:], ro2_t[:, 0:F],
                                        rcp_t[:].broadcast_to([128, F]),
                                        mybir.AluOpType.mult)
                ptf = ptr.tile([128, 128], FP, name="ptf", tag="tr")
                nc.tensor.transpose(ptf[:], hg_t[:], ident_t[:])
                nc.vector.tensor_copy(hgT_t[:], ptf[:])
                plog = pro.tile([128, F + 1], FP, name="plog", tag="ro")
                nc.tensor.matmul(plog[:, 0:CLASSES], hgT_t[:], wc_t[:], start=True, stop=True)
                nc.vector.tensor_tensor(logit_t[:], plog[:, 0:CLASSES], bcr_t[:],
                                        mybir.AluOpType.add)
                nc.sync.dma_start(out_d[:, :], logit_t[:])

    nc.finalize()
    return nc


def kernel(x, src, dst, graph_ids, W0, b0, W1, b1, W2, b2, Wc, bc, **_):
    x = np.asarray(x, np.float32)
    graph_ids = np.asarray(graph_ids, np.int64)
    idx_w, slot_cols, CA, CB, choff, NCH, TOT = _prep_edges(np.asarray(src), np.asarray(dst))

    nc = _build_program(CA, CB, choff, NCH, TOT)

    in_maps = []
    Ws = [np.asarray(W0, np.float32), np.asarray(W1, np.float32), np.asarray(W2, np.float32)]
    bs = [np.asarray(b0, np.float32), np.asarray(b1, np.float32), np.asarray(b2, np.float32)]
    b_cols = np.stack(bs, axis=1).astype(np.float32)            # [128, 3]
    bc_rep = np.tile(np.asarray(bc, np.float32)[None, :], (128, 1))
    for c in range(NCORES):
        x_loc = np.zeros((NPAD, F), np.float32)
        x_loc[:PER] = x[c * PER : (c + 1) * PER]
        gsl = np.full(NPAD, -1.0, np.float32)
        gsl[:PER] = graph_ids[c * PER : (c + 1) * PER].astype(np.float32)
        val = np.zeros(NPAD, np.float32)
        val[:PER] = 1.0
        in_maps.append(dict(
            x_loc=x_loc,
            idx_w=idx_w[c],
            slot_cols=slot_cols[c],
            gslot=gsl.reshape(GRP, 128).T.copy(),
            valid=val.reshape(GRP, 128).T.copy(),
            w0=Ws[0], w1=Ws[1], w2=Ws[2],
            b_cols=b_cols, wc=np.asarray(Wc, np.float32),
            bc_rep=bc_rep,
        ))

    res = bass_utils.run_bass_kernel_spmd(nc, in_maps, core_ids=list(range(NCORES)))
    return np.asarray(res.results[0]["out"], np.float32)
